# revision 8
# baseline (speedup 1.0000x reference)
"""TRN2 Bass kernel for nn_ClassAttention (1x1 conv + BN + ReLU + windowed attention).

kernel(**inputs) takes FULL inputs, returns the FULL output [4,256,256,256] f32.
Shards data-parallel over (batch, image-row-half) across 8 NeuronCores, runs a
Bass/Tile SPMD program via run_bass_kernel_spmd, and unshards on the host.

All device I/O is bf16 (inputs rounded on host; output staged bf16 and upcast on
host) — halves HBM traffic vs f32 and runs matmuls at full bf16 PE rate. PSUM
accumulation stays f32. Abs tolerance budget (rel < 2e-2) comfortably covers
bf16 quantization (~4e-3 per element).

Per-core shard (core = (b, rh) = (core//2, core%2)):
  x_sh   [256c, 16hh, 2048]   bf16, x[b,:,128rh:+128,:] rearranged window-
                              contiguous: [c, hh, (pw, win, r1, r2)]
  at_sh  [16hh, 128, 16384]   bf16, attn pre-transposed [pair, 64*win+k,
                              64*nh+q], partition-major per row of windows
  w_prep [256c, 256o]         bf16, (w_conv * inv_std[:,None]).T (BN folded)
  bias_r [1, 256]             bf16, (beta - mean*inv_std) row for the K=1
                              bias matmul
  out    [16hh, 128p, 4096]   bf16 staging dump; host upcasts + decodes
                              p = 64hl+8r1+r2, f = pw*256+j*32+win*16+d,
                              ch = 32j+16hl+d

On-chip pipeline per window-pair (2 windows of 64 pixels, pixels on partitions):
  conv (PE): psum[128pix=(win,r1,r2), 256ch] = bias + x_pair.T @ w_prep
             K=1 ones-matmul broadcasts the bias row, then 2 matmuls (K=128
             halves), all accumulated f32 in PSUM
  relu (ACT+DVE): block-diagonal V [128, (nh,win,d)] bf16: win0 rows via ACT
             relu, win1 rows via DVE tensor_scalar_max — splits elementwise
             load across both engines; off-diag cells stay zero (zeroed once)
  attn (PE): per head-PAIR j (heads 2j,2j+1): ONE matmul computes both heads
             and both windows: stationary At[:, 128j:+128] (M=128=(h2,q64)),
             streaming V[:, 64j:+64] (N=64=(h2,win,d)), K=128. Off-diagonal
             (h_m != h_n) output blocks are garbage and simply not evacuated.
             8 matmuls/pair instead of 16 -> halves PE instruction overhead.
  evac (DVE): 2 copies/pair extract the diagonal blocks:
             rows 0:64 (head 2j) cols (j,0,win,d), rows 64:128 (head 2j+1)
             cols (j,1,win,d) -> staging [128=(hl,r1,r2), (pw,j,win,d)] bf16
  store (ACT hwdge ring): staging -> DRAM, 1 MiB contiguous per row of windows
"""

import numpy as np
from contextlib import ExitStack

from ml_dtypes import bfloat16 as np_bf16

import concourse.bacc as bacc
import concourse.tile as tile
import concourse.mybir as mybir
from concourse.bass_utils import run_bass_kernel_spmd

F32 = mybir.dt.float32
BF16 = mybir.dt.bfloat16
RELU = mybir.ActivationFunctionType.Relu

EPS = 1e-5
NCORES = 8

_cached_nc = None


def _build_program(n_vbd=10, at_bufs=4, G=4):
    nc = bacc.Bacc("TRN2", target_bir_lowering=False, debug=False)

    x_d = nc.dram_tensor("x_sh", [256, 16, 2048], BF16, kind="ExternalInput")
    at_d = nc.dram_tensor("at_sh", [16, 128, 16384], BF16, kind="ExternalInput")
    wc_d = nc.dram_tensor("w_prep", [256, 256], BF16, kind="ExternalInput")
    b_d = nc.dram_tensor("bias_r", [1, 256], BF16, kind="ExternalInput")
    out_d = nc.dram_tensor("out_sh", [16, 128, 4096], BF16, kind="ExternalOutput")

    ngroups = 16 // G

    with tile.TileContext(nc) as tc, ExitStack() as ctx:
        const = ctx.enter_context(tc.tile_pool(name="const", bufs=1))
        xp = ctx.enter_context(tc.tile_pool(name="xp", bufs=2))
        atp = ctx.enter_context(tc.tile_pool(name="atp", bufs=at_bufs))
        vbdp = ctx.enter_context(tc.tile_pool(name="vbdp", bufs=1))
        stp = ctx.enter_context(tc.tile_pool(name="stp", bufs=2))
        pscp = ctx.enter_context(tc.tile_pool(name="pscp", bufs=2, space="PSUM"))
        psap = ctx.enter_context(tc.tile_pool(name="psap", bufs=4, space="PSUM"))

        w0 = const.tile([128, 256], BF16, name="w0")
        w1 = const.tile([128, 256], BF16, name="w1")
        nc.sync.dma_start(out=w0, in_=wc_d[0:128, :])
        nc.sync.dma_start(out=w1, in_=wc_d[128:256, :])
        bias_r = const.tile([1, 256], BF16, name="bias_r_t")
        nc.sync.dma_start(out=bias_r, in_=b_d[:, :])
        ones = const.tile([1, 128], BF16, name="ones_t")
        nc.vector.memset(ones, 1.0)

        # Block-diagonal V tiles: columns = (nh 16, win 2, d 16). Zeroed once;
        # the relu writes only the diagonal cells (win0 -> rows 0:64 of win-0
        # columns, win1 -> rows 64:128 of win-1 columns), so the zeros persist
        # across reuse and each V[:, 32nh:+32] is exactly block-diag(V0, V1).
        vbd = []
        for i in range(n_vbd):
            t = vbdp.tile([128, 512], BF16, tag=f"vbd{i}", name=f"vbd{i}")
            nc.vector.memset(t, 0.0)
            vbd.append(t)
        vbd_i = 0

        for hh in range(16):
            xt0 = xp.tile([128, 2048], BF16, tag="xt0", name=f"xt0_{hh}")
            xt1 = xp.tile([128, 2048], BF16, tag="xt1", name=f"xt1_{hh}")
            nc.sync.dma_start(out=xt0, in_=x_d[0:128, hh, :])
            nc.sync.dma_start(out=xt1, in_=x_d[128:256, hh, :])

            st = stp.tile([128, 4096], BF16, tag="st", name=f"st_{hh}")
            # partitions p = (hl 2, r1 8, r2 8); cols f = pw*256 + j*32 + win*16 + d
            st_r = st.rearrange("p (pw j win d) -> p pw j win d",
                                pw=16, j=8, win=2, d=16)

            for g in range(ngroups):
                at = atp.tile([128, 1024 * G], BF16, tag="at", name=f"at_{hh}_{g}")
                nc.sync.dma_start(
                    out=at,
                    in_=at_d[hh, :, 1024 * G * g: 1024 * G * (g + 1)])

                Vg = []
                for iG in range(G):
                    p8 = G * g + iG
                    ps = pscp.tile([128, 256], F32, tag="psc", name=f"ps_{hh}_{p8}")
                    xsl = slice(128 * p8, 128 * p8 + 128)
                    nc.tensor.matmul(ps, ones, bias_r, start=True, stop=False)
                    nc.tensor.matmul(ps, xt0[:, xsl], w0, start=False, stop=False)
                    nc.tensor.matmul(ps, xt1[:, xsl], w1, start=False, stop=True)
                    V = vbd[vbd_i % n_vbd]
                    vbd_i += 1
                    Vr = V.rearrange("p (nh two d) -> p nh two d", nh=16, two=2, d=16)
                    psr = ps.rearrange("p (a b) -> p a b", a=16)
                    nc.scalar.activation(Vr[0:64, :, 0, :], psr[0:64], RELU)
                    nc.vector.tensor_scalar_max(Vr[64:128, :, 1, :], psr[64:128], 0.0)
                    Vg.append(V)

                for iG in range(G):
                    p8 = G * g + iG
                    V = Vg[iG]
                    pa = psap.tile([128, 512], F32, tag="pa", name=f"pa_{hh}_{p8}")
                    for j in range(8):
                        # out[(hl,q), (hl',win,d)] = sum_k at[k,(hl,q)]*V[k,(hl',win,d)]
                        # for heads (2j, 2j+1); only hl==hl' blocks are real.
                        nc.tensor.matmul(
                            pa[:, 64 * j:64 * j + 64],
                            at[:, 1024 * iG + 128 * j: 1024 * iG + 128 * j + 128],
                            V[:, 64 * j:64 * j + 64],
                            start=True, stop=True)
                    # diagonal extraction: head 2j from rows 0:64 cols (j,0,*),
                    # head 2j+1 from rows 64:128 cols (j,1,*)
                    pa_r = pa.rearrange("p (j hl win d) -> p j hl win d",
                                        j=8, hl=2, win=2, d=16)
                    nc.vector.tensor_copy(st_r[0:64, p8], pa_r[0:64, :, 0])
                    nc.vector.tensor_copy(st_r[64:128, p8], pa_r[64:128, :, 1])

            nc.scalar.dma_start(out=out_d[hh], in_=st[:, :])

    nc.compile()
    return nc


def _shard_inputs(x, attn_i, w_conv, bn_gamma, bn_beta, bn_mean, bn_var):
    inv_std = (bn_gamma / np.sqrt(bn_var + np.float32(EPS))).astype(np.float32)
    shift = (bn_beta - bn_mean * inv_std).astype(np.float32)
    bias_r = np.ascontiguousarray(shift[None, :]).astype(np_bf16)
    w_prep = np.ascontiguousarray((w_conv * inv_std[:, None]).T).astype(np_bf16)
    in_maps = []
    for core in range(NCORES):
        b, rh = core // 2, core % 2
        x_sh = x[b, :, 128 * rh:128 * rh + 128, :].astype(np_bf16)
        x_sh = np.ascontiguousarray(
            x_sh.reshape(256, 16, 8, 16, 2, 8).transpose(0, 1, 3, 4, 2, 5)
        ).reshape(256, 16, 2048)
        a_sl = attn_i[1024 * b + 512 * rh: 1024 * b + 512 * rh + 512].astype(np_bf16)
        # [pair, 64win+k, 64nh+q], then partition-major per hh row
        # ([hh, p, pr, 1024]) so each at-load reads 8KiB/partition contiguous
        a_prep = a_sl.reshape(256, 2, 16, 64, 64).transpose(0, 1, 4, 2, 3) \
            .reshape(16, 16, 128, 1024)
        a_prep = np.ascontiguousarray(
            a_prep.transpose(0, 2, 1, 3)).reshape(16, 128, 16384)
        in_maps.append(dict(x_sh=x_sh, at_sh=a_prep, w_prep=w_prep, bias_r=bias_r))
    return in_maps


def _unshard_output(results):
    out = np.empty((4, 256, 256, 256), np.float32)
    for core in range(NCORES):
        b, rh = core // 2, core % 2
        raw = results[core]["out_sh"]  # [16, 128, 4096] bf16
        # bf16 -> f32 upcast is exact: shift the 16 payload bits up
        raw_f = (np.asarray(raw).view(np.uint16).astype(np.uint32) << 16) \
            .view(np.float32)
        r = raw_f.reshape(16, 2, 8, 8, 16, 8, 2, 16)  # hh,hl,r1,r2,pw,j,win,d
        # ch = 32j+16hl+d ; h = 8hh+r1 ; w = 16pw+8win+r2
        oc = r.transpose(5, 1, 7, 0, 2, 4, 6, 3).reshape(256, 128, 256)
        out[b, :, 128 * rh:128 * rh + 128, :] = oc
    return out


def get_program():
    global _cached_nc
    if _cached_nc is None:
        _cached_nc = _build_program()
    return _cached_nc


def run_sharded(in_maps, trace=False, **kwargs):
    nc = get_program()
    return run_bass_kernel_spmd(nc, in_maps, list(range(NCORES)),
                                trace=trace, **kwargs)


def kernel(x, attn_i, w_conv, bn_gamma, bn_beta, bn_mean, bn_var):
    x = np.asarray(x, dtype=np.float32)
    attn_i = np.asarray(attn_i, dtype=np.float32)
    w_conv = np.asarray(w_conv, dtype=np.float32)
    bn_gamma = np.asarray(bn_gamma, dtype=np.float32)
    bn_beta = np.asarray(bn_beta, dtype=np.float32)
    bn_mean = np.asarray(bn_mean, dtype=np.float32)
    bn_var = np.asarray(bn_var, dtype=np.float32)
    in_maps = _shard_inputs(x, attn_i, w_conv, bn_gamma, bn_beta, bn_mean, bn_var)
    res = run_sharded(in_maps)
    return _unshard_output(res.results)


# revision 14
# speedup vs baseline: 1.1240x; 1.1240x over previous
"""TRN2 Bass kernel for nn_ClassAttention (1x1 conv + BN + ReLU + windowed attention).

kernel(**inputs) takes FULL inputs, returns the FULL output [4,256,256,256] f32.
Shards data-parallel over (batch, image-row-half) across 8 NeuronCores, runs a
Bass/Tile SPMD program via run_bass_kernel_spmd, and unshards on the host.

All device I/O is bf16 (inputs rounded on host; output staged bf16 and upcast on
host) — halves HBM traffic vs f32 and runs matmuls at full bf16 PE rate. PSUM
accumulation stays f32. Abs tolerance budget (rel < 2e-2) comfortably covers
bf16 quantization (~4e-3 per element).

Per-core shard (core = (b, rh) = (core//2, core%2)):
  x_sh   [256c, 16hh, 2048]   bf16, x[b,:,128rh:+128,:] rearranged window-
                              contiguous: [c, hh, (pw, win, r1, r2)]
  at_sh  [16hh, 128, 16384]   bf16, attn pre-transposed [pair, 64*win+k,
                              64*nh+q], partition-major per row of windows
  w_prep [256c, 256o]         bf16, (w_conv * inv_std[:,None]).T (BN folded)
  bias_r [1, 256]             bf16, (beta - mean*inv_std) row for the K=1
                              bias matmul
  out    [16hh, 128p, 4096]   bf16 staging dump; host upcasts + decodes
                              p = 32q+16win+d, f = u*1024+r1*128+pw*8+r2,
                              ch = 64u+16q+d

On-chip pipeline per window-pair (2 windows of 64 pixels, pixels on partitions):
  conv (PE): psum[128pix=(win,r1,r2), 256ch] = bias + x_pair.T @ w_prep
             K=1 ones-matmul broadcasts the bias row, then 2 matmuls (K=128
             halves), all accumulated f32 in PSUM
  relu (ACT+DVE): block-diagonal V [128, (nh,win,d)] bf16: win0 rows via ACT
             relu, win1 rows via DVE tensor_scalar_max — splits elementwise
             load across both engines; off-diag cells stay zero (zeroed once)
  attn (PE): per head nh: one matmul computes BOTH windows via block-diag V:
             out[32,64] = V[:,32nh:+32].T @ At[:,64nh:+64], K=128, N=64,
             tile_position=(0, 32*(nh%4)) -> 4 column-tiles packed in the array
  evac (DVE): psum [128,(u,r1,r2)] f32 -> staging [128, 4096] bf16
  store (ACT hwdge ring): staging -> DRAM, 1 MiB contiguous per row of windows
"""

import numpy as np
from contextlib import ExitStack

from ml_dtypes import bfloat16 as np_bf16

import concourse.bacc as bacc
import concourse.tile as tile
import concourse.mybir as mybir
from concourse.bass_utils import run_bass_kernel_spmd

F32 = mybir.dt.float32
BF16 = mybir.dt.bfloat16
RELU = mybir.ActivationFunctionType.Relu

EPS = 1e-5
NCORES = 8

_cached_nc = None


def _build_program(n_vbd=10, at_bufs=4, G=4):
    nc = bacc.Bacc("TRN2", target_bir_lowering=False, debug=False)

    x_d = nc.dram_tensor("x_sh", [256, 16, 2048], BF16, kind="ExternalInput")
    at_d = nc.dram_tensor("at_sh", [16, 128, 16384], BF16, kind="ExternalInput")
    wc_d = nc.dram_tensor("w_prep", [256, 256], BF16, kind="ExternalInput")
    b_d = nc.dram_tensor("bias_r", [1, 256], BF16, kind="ExternalInput")
    out_d = nc.dram_tensor("out_sh", [16, 128, 4096], BF16, kind="ExternalOutput")

    ngroups = 16 // G

    with tile.TileContext(nc) as tc, ExitStack() as ctx:
        const = ctx.enter_context(tc.tile_pool(name="const", bufs=1))
        xp = ctx.enter_context(tc.tile_pool(name="xp", bufs=3))
        atp = ctx.enter_context(tc.tile_pool(name="atp", bufs=at_bufs))
        vbdp = ctx.enter_context(tc.tile_pool(name="vbdp", bufs=1))
        stp = ctx.enter_context(tc.tile_pool(name="stp", bufs=3))
        pscp = ctx.enter_context(tc.tile_pool(name="pscp", bufs=3, space="PSUM"))
        psap = ctx.enter_context(tc.tile_pool(name="psap", bufs=5, space="PSUM"))

        w0 = const.tile([128, 256], BF16, name="w0")
        w1 = const.tile([128, 256], BF16, name="w1")
        nc.sync.dma_start(out=w0, in_=wc_d[0:128, :])
        nc.sync.dma_start(out=w1, in_=wc_d[128:256, :])
        bias_r = const.tile([1, 256], BF16, name="bias_r_t")
        nc.sync.dma_start(out=bias_r, in_=b_d[:, :])
        ones = const.tile([1, 128], BF16, name="ones_t")
        nc.vector.memset(ones, 1.0)

        # Block-diagonal V tiles: columns = (nh 16, win 2, d 16). Zeroed once;
        # the relu writes only the diagonal cells (win0 -> rows 0:64 of win-0
        # columns, win1 -> rows 64:128 of win-1 columns), so the zeros persist
        # across reuse and each V[:, 32nh:+32] is exactly block-diag(V0, V1).
        vbd = []
        for i in range(n_vbd):
            t = vbdp.tile([128, 512], BF16, tag=f"vbd{i}", name=f"vbd{i}")
            nc.vector.memset(t, 0.0)
            vbd.append(t)
        vbd_i = 0

        for hh in range(16):
            xt0 = xp.tile([128, 2048], BF16, tag="xt0", name=f"xt0_{hh}")
            xt1 = xp.tile([128, 2048], BF16, tag="xt1", name=f"xt1_{hh}")
            nc.sync.dma_start(out=xt0, in_=x_d[0:128, hh, :])
            nc.sync.dma_start(out=xt1, in_=x_d[128:256, hh, :])

            st = stp.tile([128, 4096], BF16, tag="st", name=f"st_{hh}")
            # f = u*1024 + r1*128 + pw*8 + r2
            st_r = st.rearrange("p (u r1 pw r2) -> p pw u r1 r2",
                                u=4, r1=8, pw=16, r2=8)

            for g in range(ngroups):
                at = atp.tile([128, 1024 * G], BF16, tag="at", name=f"at_{hh}_{g}")
                nc.sync.dma_start(
                    out=at,
                    in_=at_d[hh, :, 1024 * G * g: 1024 * G * (g + 1)])

                Vg = []
                for iG in range(G):
                    p8 = G * g + iG
                    ps = pscp.tile([128, 256], F32, tag="psc", name=f"ps_{hh}_{p8}")
                    xsl = slice(128 * p8, 128 * p8 + 128)
                    nc.tensor.matmul(ps, ones, bias_r, start=True, stop=False)
                    nc.tensor.matmul(ps, xt0[:, xsl], w0, start=False, stop=False)
                    nc.tensor.matmul(ps, xt1[:, xsl], w1, start=False, stop=True)
                    V = vbd[vbd_i % n_vbd]
                    vbd_i += 1
                    Vr = V.rearrange("p (nh two d) -> p nh two d", nh=16, two=2, d=16)
                    psr = ps.rearrange("p (a b) -> p a b", a=16)
                    nc.scalar.activation(Vr[0:64, :, 0, :], psr[0:64], RELU)
                    nc.vector.tensor_scalar_max(Vr[64:128, :, 1, :], psr[64:128], 0.0)
                    Vg.append(V)

                for iG in range(G):
                    p8 = G * g + iG
                    V = Vg[iG]
                    pa = psap.tile([128, 256], F32, tag="pa", name=f"pa_{hh}_{p8}")
                    for j in range(4):
                        for quad in range(4):
                            nh = 4 * j + quad
                            nc.tensor.matmul(
                                pa[32 * quad:32 * quad + 32, 64 * j:64 * j + 64],
                                V[:, 32 * nh:32 * nh + 32],
                                at[:, 1024 * iG + 64 * nh: 1024 * iG + 64 * nh + 64],
                                start=True, stop=True,
                                tile_position=(0, 32 * quad))
                    src = pa.rearrange("p (u r1 r2) -> p u r1 r2", u=4, r1=8, r2=8)
                    nc.vector.tensor_copy(st_r[:, p8], src)

            nc.scalar.dma_start(out=out_d[hh], in_=st[:, :])

    nc.compile()
    return nc


def _shard_inputs(x, attn_i, w_conv, bn_gamma, bn_beta, bn_mean, bn_var):
    inv_std = (bn_gamma / np.sqrt(bn_var + np.float32(EPS))).astype(np.float32)
    shift = (bn_beta - bn_mean * inv_std).astype(np.float32)
    bias_r = np.ascontiguousarray(shift[None, :]).astype(np_bf16)
    w_prep = np.ascontiguousarray((w_conv * inv_std[:, None]).T).astype(np_bf16)
    in_maps = []
    for core in range(NCORES):
        b, rh = core // 2, core % 2
        x_sh = x[b, :, 128 * rh:128 * rh + 128, :].astype(np_bf16)
        x_sh = np.ascontiguousarray(
            x_sh.reshape(256, 16, 8, 16, 2, 8).transpose(0, 1, 3, 4, 2, 5)
        ).reshape(256, 16, 2048)
        a_sl = attn_i[1024 * b + 512 * rh: 1024 * b + 512 * rh + 512].astype(np_bf16)
        # [pair, 64win+k, 64nh+q], then partition-major per hh row
        # ([hh, p, pr, 1024]) so each at-load reads 8KiB/partition contiguous
        a_prep = a_sl.reshape(256, 2, 16, 64, 64).transpose(0, 1, 4, 2, 3) \
            .reshape(16, 16, 128, 1024)
        a_prep = np.ascontiguousarray(
            a_prep.transpose(0, 2, 1, 3)).reshape(16, 128, 16384)
        in_maps.append(dict(x_sh=x_sh, at_sh=a_prep, w_prep=w_prep, bias_r=bias_r))
    return in_maps


def _unshard_output(results):
    out = np.empty((4, 256, 256, 256), np.float32)
    for core in range(NCORES):
        b, rh = core // 2, core % 2
        raw = results[core]["out_sh"]  # [16, 128, 4096] bf16
        # bf16 -> f32 upcast is exact: shift the 16 payload bits up
        raw_f = (np.asarray(raw).view(np.uint16).astype(np.uint32) << 16) \
            .view(np.float32)
        r = raw_f.reshape(16, 4, 2, 16, 4, 8, 16, 8)  # hh,q,win,d,u,r1,pw,r2
        # ch = 64u+16q+d ; h = 8hh+r1 ; w = 16pw+8win+r2
        oc = r.transpose(4, 1, 3, 0, 5, 6, 2, 7).reshape(256, 128, 256)
        out[b, :, 128 * rh:128 * rh + 128, :] = oc
    return out


def get_program():
    global _cached_nc
    if _cached_nc is None:
        _cached_nc = _build_program()
    return _cached_nc


def run_sharded(in_maps, trace=False, **kwargs):
    nc = get_program()
    return run_bass_kernel_spmd(nc, in_maps, list(range(NCORES)),
                                trace=trace, **kwargs)


def kernel(x, attn_i, w_conv, bn_gamma, bn_beta, bn_mean, bn_var):
    x = np.asarray(x, dtype=np.float32)
    attn_i = np.asarray(attn_i, dtype=np.float32)
    w_conv = np.asarray(w_conv, dtype=np.float32)
    bn_gamma = np.asarray(bn_gamma, dtype=np.float32)
    bn_beta = np.asarray(bn_beta, dtype=np.float32)
    bn_mean = np.asarray(bn_mean, dtype=np.float32)
    bn_var = np.asarray(bn_var, dtype=np.float32)
    in_maps = _shard_inputs(x, attn_i, w_conv, bn_gamma, bn_beta, bn_mean, bn_var)
    res = run_sharded(in_maps)
    return _unshard_output(res.results)
